# revision 10
# baseline (speedup 1.0000x reference)
"""KAN layer kernel for 8 Trainium2 NeuronCores.

Math (reference):
    basis[b,i] = sum_h silu(x[b,i]*w1[i%K,h] + b1[i%K,h]) * w2[i%K,h] + b2[i%K]
    out[b,o]   = sum_i basis[b,i] * Wsum[o,i],   Wsum = W.sum(-1)   # [O,I]

Sharding: data-parallel over the input-feature axis I (16384 -> 8 x 2048).
Each core computes a partial out[64,1024] over its feature slice; host sums.

The kernel is memory-bound on reading W (335 MB fp32). Key ideas vs the
accum-DMA baseline (204.6 us):
  - W is host-cast to fp16 (tolerance is 2e-2; fp16 W contributes ~1e-3
    rel err), halving HBM traffic to ~21 MB/core -> ~59 us DMA floor.
  - The k-reduction (Wsum = W.sum(-1)) happens in the PE: for each i-tile
    the 5 k-slices are 5 accumulating matmuls into the same PSUM banks.
    The DMA stream is then pure bypass reads at full bandwidth (no
    SBUF read-modify-write serialization like accum_op=add had).
  - Features are sorted by k=i%5 on the host so each SBUF partition row
    holds 16 features of a single k. ACT then evaluates silu over all
    16 i-tiles in ONE instruction per h with per-partition scale/bias
    ([128,512] x2 halves instead of 16x[128,64]), cutting ACT instruction
    overhead ~5x so basis compute hides fully under the W stream.
    The <=1 row per core where k changes mid-row goes to a separate
    16-partition "mixed" block with its own tiny ACT/DVE/matmul path.
"""
import numpy as np

B, I, O, K, H = 64, 16384, 1024, 5, 16
NCORES = 8
IC = I // NCORES          # 2048 features per core
P = 128                   # partitions
NT = IC // P              # 16 i-tiles per core
NB = B                    # 64
NO = O                    # 1024
KO = K * NO               # 5120 cols of W per feature
XW = NT * NB + NB         # x block cols: main grid + mixed block
PRC = 2 * (3 * H + 1)     # param cols: main w1,b1,w2,b2 + mixed w1,b1,w2,b2
MO = 3 * H + 1            # offset of mixed params
WCHUNKS = [2, 2, 2, 2, 2, 2, 2, 2]      # W stream transfer sizes (tiles)

TRACE = False             # test.py sets True to capture an NTFF profile
LAST_RESULT = None


def _build():
    from contextlib import ExitStack
    from concourse import bacc, mybir, tile

    f32 = mybir.dt.float32
    f16 = mybir.dt.float16
    nc = bacc.Bacc("TRN2", target_bir_lowering=False, debug=False,
                   num_devices=NCORES)
    Wg = nc.declare_dram_parameter("Wg", [P, NT * KO], f16, isOutput=False)
    Wmd = nc.declare_dram_parameter("Wm", [H, KO], f16, isOutput=False)
    xbd = nc.declare_dram_parameter("xb", [P, XW], f16, isOutput=False)
    prd = nc.declare_dram_parameter("pr", [P, PRC], f32, isOutput=False)
    # out rows 0:64 = column-group-0 partial, rows 64:128 = group-1 partial;
    # the host adds them (it sums the 8 per-core partials anyway).
    out = nc.declare_dram_parameter("out", [P, NO], f32, isOutput=True)

    with tile.TileContext(nc) as tc, ExitStack() as ctx:
        const = ctx.enter_context(tc.tile_pool(name="const", bufs=1))
        wpool = ctx.enter_context(tc.tile_pool(name="w", bufs=5))
        spool = ctx.enter_context(tc.tile_pool(name="silu", bufs=3))
        smpool = ctx.enter_context(tc.tile_pool(name="silum", bufs=2))
        apool = ctx.enter_context(tc.tile_pool(name="acc", bufs=1))
        opool = ctx.enter_context(tc.tile_pool(name="out", bufs=1))
        psum = ctx.enter_context(tc.tile_pool(name="psum", bufs=1, space="PSUM"))

        # const loads ride the (otherwise idle) gpsimd SWDGE queue so the
        # W stream owns the sync HWDGE queue from t=0.
        xb = const.tile([P, XW], f16)
        pr = const.tile([P, PRC], f32)
        wm = const.tile([H, KO], f16)
        nc.gpsimd.dma_start(xb[:, :], xbd[:, :])
        nc.gpsimd.dma_start(pr[:, :], prd[:, :])
        nc.gpsimd.dma_start(wm[:, :], Wmd[:, :])

        # Two PSUM banks, each split into two 64-partition accumulation
        # regions: [0:64] serves PE column group 0, [64:128] group 1.
        ps0 = psum.tile([P, 512], f32, tag="ps0")
        ps1 = psum.tile([P, 512], f32, tag="ps1")

        Silu = mybir.ActivationFunctionType.Silu
        mult, add = mybir.AluOpType.mult, mybir.AluOpType.add

        # ---- basisT[i,b] for the main grid, two halves (tiles 0-7, 8-15)
        # so matmuls on half A can start while half B still computes.
        acc = apool.tile([P, NT * NB], f32)
        acc16 = apool.tile([P, NT * NB], f16)
        XH = NT * NB // 2
        for half in range(2):
            xs = xb[:, half * XH:(half + 1) * XH]
            for h in range(H):
                st = spool.tile([P, XH], f32)
                nc.scalar.activation(
                    st[:, :], xs, Silu,
                    bias=pr[:, H + h:H + h + 1], scale=pr[:, h:h + 1])
                asl = acc[:, half * XH:(half + 1) * XH]
                if h == 0:
                    nc.vector.tensor_scalar(
                        asl, st[:, :], pr[:, 2 * H:2 * H + 1],
                        pr[:, 3 * H:3 * H + 1], op0=mult, op1=add)
                elif h < H - 1:
                    nc.vector.scalar_tensor_tensor(
                        asl, st[:, :], pr[:, 2 * H + h:2 * H + h + 1], asl,
                        op0=mult, op1=add)
                else:
                    nc.vector.scalar_tensor_tensor(
                        acc16[:, half * XH:(half + 1) * XH], st[:, :],
                        pr[:, 2 * H + h:2 * H + h + 1], asl,
                        op0=mult, op1=add)

        # ---- mixed block: 16 features whose k varies within the row;
        # one feature per partition, so per-partition scale handles any k.
        accm = apool.tile([H, NB], f32)
        accm16 = apool.tile([H, NB], f16)
        xm = xb[0:H, NT * NB:NT * NB + NB]
        for h in range(H):
            stm = smpool.tile([H, NB], f32)
            nc.scalar.activation(
                stm[:, :], xm, Silu,
                bias=pr[0:H, MO + H + h:MO + H + h + 1],
                scale=pr[0:H, MO + h:MO + h + 1])
            if h == 0:
                nc.vector.tensor_scalar(
                    accm[:, :], stm[:, :], pr[0:H, MO + 2 * H:MO + 2 * H + 1],
                    pr[0:H, MO + 3 * H:MO + 3 * H + 1], op0=mult, op1=add)
            elif h < H - 1:
                nc.vector.scalar_tensor_tensor(
                    accm[:, :], stm[:, :],
                    pr[0:H, MO + 2 * H + h:MO + 2 * H + h + 1], accm[:, :],
                    op0=mult, op1=add)
            else:
                nc.vector.scalar_tensor_tensor(
                    accm16[:, :], stm[:, :],
                    pr[0:H, MO + 2 * H + h:MO + 2 * H + h + 1], accm[:, :],
                    op0=mult, op1=add)

        # ---- W stream + matmuls: out[b,o] += basisT.T @ W[:,k,:] per k,
        # k-sum via PSUM accumulation. The two tiles of each chunk run
        # CONCURRENTLY in the PE via disjoint column groups (tile_position
        # (0,0) vs (0,64)), doubling effective matmul throughput; the two
        # halves are summed on DVE at the end.
        started = set()
        NC_ = len(WCHUNKS)

        def mm(reg, out_ap, lhsT, rhs, tp, last=False):
            nc.tensor.matmul(out_ap, lhsT, rhs, start=reg not in started,
                             stop=last, tile_position=tp)
            started.add(reg)

        t0 = 0
        for ci, cw in enumerate(WCHUNKS):
            assert cw == 2
            wt = wpool.tile([P, cw * KO], f16, tag="w", name=f"w{ci}")
            nc.sync.dma_start(wt[:, :], Wg[:, t0 * KO:(t0 + cw) * KO])
            lA = acc16[:, t0 * NB:(t0 + 1) * NB]
            lB = acc16[:, (t0 + 1) * NB:(t0 + 2) * NB]
            for k in range(K):
                rb = k * NO
                e = ci == NC_ - 1 and k == K - 1
                mm("00", ps0[0:NB, :], lA, wt[:, rb:rb + 512], (0, 0))
                mm("01", ps0[NB:P, :], lB, wt[:, KO + rb:KO + rb + 512],
                   (0, NB), last=e)
                mm("10", ps1[0:NB, :], lA, wt[:, rb + 512:rb + NO], (0, 0),
                   last=e)
                mm("11", ps1[NB:P, :], lB, wt[:, KO + rb + 512:KO + rb + NO],
                   (0, NB))
            t0 += cw

        for k in range(K):
            e = k == K - 1
            rb = k * NO
            mm("00", ps0[0:NB, :], accm16[:, :], wm[:, rb:rb + 512], (0, 0),
               last=e)
            mm("11", ps1[NB:P, :], accm16[:, :], wm[:, rb + 512:rb + NO],
               (0, NB), last=e)

        # evacuate PSUM; the two halves are copied/stored independently so
        # DVE and DMA overlap. Column-group halves are summed on the host
        # (DVE cannot add across partitions).
        out_sb = opool.tile([P, NO], f32)
        nc.vector.tensor_copy(out_sb[:, 0:512], ps0[:, :])
        nc.sync.dma_start(out[:, 0:512], out_sb[:, 0:512])
        nc.vector.tensor_copy(out_sb[:, 512:1024], ps1[:, :])
        nc.sync.dma_start(out[:, 512:1024], out_sb[:, 512:1024])
    nc.compile()
    return nc


def kernel(x, w1, b1, w2, b2, W):
    global LAST_RESULT
    from concourse.bass_utils import run_bass_kernel_spmd

    x = np.asarray(x, dtype=np.float32)
    W = np.asarray(W, dtype=np.float32)
    w1 = np.asarray(w1, dtype=np.float32)
    b1 = np.asarray(b1, dtype=np.float32)
    w2 = np.asarray(w2, dtype=np.float32)
    b2 = np.asarray(b2, dtype=np.float32)

    # ---- host prep: sort features by k so each SBUF partition row holds
    # 16 same-k features; transpose W to contraction-major [i,k,o] fp16.
    idxk = np.arange(I) % K
    perm = np.argsort(idxk, kind="stable")
    xT = np.ascontiguousarray(x.T)                          # [I, B]
    W16t = np.ascontiguousarray(
        W.astype(np.float16).reshape(O, I * K).T).reshape(I, K, O)

    in_maps = []
    for c in range(NCORES):
        fc = perm[c * IC:(c + 1) * IC]
        rows = fc.reshape(P, NT)                            # row = 16 features
        rowk = idxk[rows]
        mixed = (rowk != rowk[:, :1]).any(axis=1)
        assert int(mixed.sum()) <= 1
        rbi = int(np.argmax(mixed)) if mixed.any() else P - 1
        keep = np.array([r for r in range(P) if r != rbi])
        featPT = rows[keep]                                 # [127, NT]
        mixf = rows[rbi]                                    # [16]

        xbh = np.zeros((P, XW), dtype=np.float16)
        xbh[:P - 1, :NT * NB] = xT[featPT].reshape(P - 1, NT * NB)
        xbh[:H, NT * NB:] = xT[mixf]

        prh = np.zeros((P, PRC), dtype=np.float32)
        rk = rowk[keep][:, 0]
        prh[:P - 1, 0:H] = w1[rk]
        prh[:P - 1, H:2 * H] = b1[rk]
        prh[:P - 1, 2 * H:3 * H] = w2[rk]
        prh[:P - 1, 3 * H] = b2[rk]
        mk = idxk[mixf]
        prh[:H, MO:MO + H] = w1[mk]
        prh[:H, MO + H:MO + 2 * H] = b1[mk]
        prh[:H, MO + 2 * H:MO + 3 * H] = w2[mk]
        prh[:H, MO + 3 * H] = b2[mk]

        Wgh = np.zeros((P, NT * KO), dtype=np.float16)
        Wgh[:P - 1] = W16t[featPT].reshape(P - 1, NT * KO)
        Wmh = np.ascontiguousarray(W16t[mixf].reshape(H, KO))

        in_maps.append({"Wg": Wgh, "Wm": Wmh, "xb": xbh, "pr": prh})

    nc = _build()
    res = run_bass_kernel_spmd(nc, in_maps, list(range(NCORES)), trace=TRACE)
    LAST_RESULT = res
    outv = np.zeros((B, O), dtype=np.float32)
    for c in range(NCORES):
        oc = res.results[c]["out"]
        outv += oc[0:NB] + oc[NB:P]
    return outv


# revision 12
# speedup vs baseline: 1.1443x; 1.1443x over previous
"""KAN layer kernel for 8 Trainium2 NeuronCores.

Math (reference):
    basis[b,i] = sum_h silu(x[b,i]*w1[i%K,h] + b1[i%K,h]) * w2[i%K,h] + b2[i%K]
    out[b,o]   = sum_i basis[b,i] * Wsum[o,i],   Wsum = W.sum(-1)   # [O,I]

Sharding: data-parallel over the input-feature axis I (16384 -> 8 x 2048).
Each core computes a partial out[64,1024] over its feature slice; host sums.

The kernel is memory-bound on reading W (335 MB fp32). Key ideas vs the
accum-DMA baseline (204.6 us):
  - W is host-cast to fp16 (tolerance is 2e-2; fp16 W contributes ~1e-3
    rel err), halving HBM traffic to ~21 MB/core -> ~59 us DMA floor.
  - The k-reduction (Wsum = W.sum(-1)) happens in the PE: for each i-tile
    the 5 k-slices are 5 accumulating matmuls into the same PSUM banks.
    The DMA stream is then pure bypass reads at full bandwidth (no
    SBUF read-modify-write serialization like accum_op=add had).
  - Features are sorted by k=i%5 on the host so each SBUF partition row
    holds 16 features of a single k. ACT then evaluates silu over all
    16 i-tiles in ONE instruction per h with per-partition scale/bias
    ([128,512] x2 halves instead of 16x[128,64]), cutting ACT instruction
    overhead ~5x so basis compute hides fully under the W stream.
    The <=1 row per core where k changes mid-row goes to a separate
    16-partition "mixed" block with its own tiny ACT/DVE/matmul path.
"""
import numpy as np

B, I, O, K, H = 64, 16384, 1024, 5, 16
NCORES = 8
IC = I // NCORES          # 2048 features per core
P = 128                   # partitions
NT = IC // P              # 16 i-tiles per core
NB = B                    # 64
NO = O                    # 1024
KO = K * NO               # 5120 cols of W per feature
XW = NT * NB + NB         # x block cols: main grid + mixed block
PRC = 2 * (3 * H + 1)     # param cols: main w1,b1,w2,b2 + mixed w1,b1,w2,b2
MO = 3 * H + 1            # offset of mixed params
WCHUNKS = [2, 2, 2, 2, 2, 2, 2, 2]      # W stream transfer sizes (tiles)

TRACE = False             # test.py sets True to capture an NTFF profile
LAST_RESULT = None


def _build():
    from contextlib import ExitStack
    from concourse import bacc, mybir, tile

    f32 = mybir.dt.float32
    f16 = mybir.dt.float16
    nc = bacc.Bacc("TRN2", target_bir_lowering=False, debug=False,
                   num_devices=NCORES)
    Wg = nc.declare_dram_parameter("Wg", [P, NT * KO], f16, isOutput=False)
    Wmd = nc.declare_dram_parameter("Wm", [H, KO], f16, isOutput=False)
    xbd = nc.declare_dram_parameter("xb", [P, XW], f16, isOutput=False)
    prd = nc.declare_dram_parameter("pr", [P, PRC], f32, isOutput=False)
    # out rows 0:64 = column-group-0 partial, rows 64:128 = group-1 partial;
    # the host adds them (it sums the 8 per-core partials anyway).
    out = nc.declare_dram_parameter("out", [P, NO], f32, isOutput=True)

    with tile.TileContext(nc) as tc, ExitStack() as ctx:
        const = ctx.enter_context(tc.tile_pool(name="const", bufs=1))
        wpool = ctx.enter_context(tc.tile_pool(name="w", bufs=5))
        spool = ctx.enter_context(tc.tile_pool(name="silu", bufs=3))
        smpool = ctx.enter_context(tc.tile_pool(name="silum", bufs=2))
        apool = ctx.enter_context(tc.tile_pool(name="acc", bufs=1))
        opool = ctx.enter_context(tc.tile_pool(name="out", bufs=1))
        psum = ctx.enter_context(tc.tile_pool(name="psum", bufs=1, space="PSUM"))

        # All DMAs ride the sync HWDGE queue: a second (SWDGE) queue makes
        # the SDMA engines round-robin at packet granularity, degrading the
        # W stream ~20%. xb/pr go first (ACT needs them); wm is only needed
        # by the final mixed matmuls so it loads after the W stream.
        xb = const.tile([P, XW], f16)
        pr = const.tile([P, PRC], f32)
        wm = const.tile([H, KO], f16)
        nc.sync.dma_start(xb[:, :], xbd[:, :])
        nc.sync.dma_start(pr[:, :], prd[:, :])

        # Two PSUM banks, each split into two 64-partition accumulation
        # regions: [0:64] serves PE column group 0, [64:128] group 1.
        ps0 = psum.tile([P, 512], f32, tag="ps0")
        ps1 = psum.tile([P, 512], f32, tag="ps1")

        Silu = mybir.ActivationFunctionType.Silu
        mult, add = mybir.AluOpType.mult, mybir.AluOpType.add

        # ---- basisT[i,b] for the main grid, two halves (tiles 0-7, 8-15)
        # so matmuls on half A can start while half B still computes.
        acc = apool.tile([P, NT * NB], f32)
        acc16 = apool.tile([P, NT * NB], f16)
        XH = NT * NB // 2
        for half in range(2):
            xs = xb[:, half * XH:(half + 1) * XH]
            for h in range(H):
                st = spool.tile([P, XH], f32)
                nc.scalar.activation(
                    st[:, :], xs, Silu,
                    bias=pr[:, H + h:H + h + 1], scale=pr[:, h:h + 1])
                asl = acc[:, half * XH:(half + 1) * XH]
                if h == 0:
                    nc.vector.tensor_scalar(
                        asl, st[:, :], pr[:, 2 * H:2 * H + 1],
                        pr[:, 3 * H:3 * H + 1], op0=mult, op1=add)
                elif h < H - 1:
                    nc.vector.scalar_tensor_tensor(
                        asl, st[:, :], pr[:, 2 * H + h:2 * H + h + 1], asl,
                        op0=mult, op1=add)
                else:
                    nc.vector.scalar_tensor_tensor(
                        acc16[:, half * XH:(half + 1) * XH], st[:, :],
                        pr[:, 2 * H + h:2 * H + h + 1], asl,
                        op0=mult, op1=add)

        # ---- mixed block: 16 features whose k varies within the row;
        # one feature per partition, so per-partition scale handles any k.
        accm = apool.tile([H, NB], f32)
        accm16 = apool.tile([H, NB], f16)
        xm = xb[0:H, NT * NB:NT * NB + NB]
        for h in range(H):
            stm = smpool.tile([H, NB], f32)
            nc.scalar.activation(
                stm[:, :], xm, Silu,
                bias=pr[0:H, MO + H + h:MO + H + h + 1],
                scale=pr[0:H, MO + h:MO + h + 1])
            if h == 0:
                nc.vector.tensor_scalar(
                    accm[:, :], stm[:, :], pr[0:H, MO + 2 * H:MO + 2 * H + 1],
                    pr[0:H, MO + 3 * H:MO + 3 * H + 1], op0=mult, op1=add)
            elif h < H - 1:
                nc.vector.scalar_tensor_tensor(
                    accm[:, :], stm[:, :],
                    pr[0:H, MO + 2 * H + h:MO + 2 * H + h + 1], accm[:, :],
                    op0=mult, op1=add)
            else:
                nc.vector.scalar_tensor_tensor(
                    accm16[:, :], stm[:, :],
                    pr[0:H, MO + 2 * H + h:MO + 2 * H + h + 1], accm[:, :],
                    op0=mult, op1=add)

        # ---- W stream + matmuls: out[b,o] += basisT.T @ W[:,k,:] per k,
        # k-sum via PSUM accumulation. The two tiles of each chunk run
        # CONCURRENTLY in the PE via disjoint column groups (tile_position
        # (0,0) vs (0,64)), doubling effective matmul throughput; the two
        # halves are summed on DVE at the end.
        started = set()
        NC_ = len(WCHUNKS)

        def mm(reg, out_ap, lhsT, rhs, tp, last=False):
            nc.tensor.matmul(out_ap, lhsT, rhs, start=reg not in started,
                             stop=last, tile_position=tp)
            started.add(reg)

        t0 = 0
        for ci, cw in enumerate(WCHUNKS):
            assert cw == 2
            wt = wpool.tile([P, cw * KO], f16, tag="w", name=f"w{ci}")
            nc.sync.dma_start(wt[:, :], Wg[:, t0 * KO:(t0 + cw) * KO])
            lA = acc16[:, t0 * NB:(t0 + 1) * NB]
            lB = acc16[:, (t0 + 1) * NB:(t0 + 2) * NB]
            for k in range(K):
                rb = k * NO
                e = ci == NC_ - 1 and k == K - 1
                mm("00", ps0[0:NB, :], lA, wt[:, rb:rb + 512], (0, 0))
                mm("01", ps0[NB:P, :], lB, wt[:, KO + rb:KO + rb + 512],
                   (0, NB), last=e)
                mm("10", ps1[0:NB, :], lA, wt[:, rb + 512:rb + NO], (0, 0),
                   last=e)
                mm("11", ps1[NB:P, :], lB, wt[:, KO + rb + 512:KO + rb + NO],
                   (0, NB))
            t0 += cw

        nc.sync.dma_start(wm[:, :], Wmd[:, :])
        for k in range(K):
            e = k == K - 1
            rb = k * NO
            mm("00", ps0[0:NB, :], accm16[:, :], wm[:, rb:rb + 512], (0, 0),
               last=e)
            mm("11", ps1[NB:P, :], accm16[:, :], wm[:, rb + 512:rb + NO],
               (0, NB), last=e)

        # evacuate PSUM; the two halves are copied/stored independently so
        # DVE and DMA overlap. Column-group halves are summed on the host
        # (DVE cannot add across partitions).
        out_sb = opool.tile([P, NO], f32)
        nc.vector.tensor_copy(out_sb[:, 0:512], ps0[:, :])
        nc.sync.dma_start(out[:, 0:512], out_sb[:, 0:512])
        nc.vector.tensor_copy(out_sb[:, 512:1024], ps1[:, :])
        nc.sync.dma_start(out[:, 512:1024], out_sb[:, 512:1024])
    nc.compile()
    return nc


def kernel(x, w1, b1, w2, b2, W):
    global LAST_RESULT
    from concourse.bass_utils import run_bass_kernel_spmd

    x = np.asarray(x, dtype=np.float32)
    W = np.asarray(W, dtype=np.float32)
    w1 = np.asarray(w1, dtype=np.float32)
    b1 = np.asarray(b1, dtype=np.float32)
    w2 = np.asarray(w2, dtype=np.float32)
    b2 = np.asarray(b2, dtype=np.float32)

    # ---- host prep: sort features by k so each SBUF partition row holds
    # 16 same-k features; transpose W to contraction-major [i,k,o] fp16.
    idxk = np.arange(I) % K
    perm = np.argsort(idxk, kind="stable")
    xT = np.ascontiguousarray(x.T)                          # [I, B]
    W16t = np.ascontiguousarray(
        W.astype(np.float16).reshape(O, I * K).T).reshape(I, K, O)

    in_maps = []
    for c in range(NCORES):
        fc = perm[c * IC:(c + 1) * IC]
        rows = fc.reshape(P, NT)                            # row = 16 features
        rowk = idxk[rows]
        mixed = (rowk != rowk[:, :1]).any(axis=1)
        assert int(mixed.sum()) <= 1
        rbi = int(np.argmax(mixed)) if mixed.any() else P - 1
        keep = np.array([r for r in range(P) if r != rbi])
        featPT = rows[keep]                                 # [127, NT]
        mixf = rows[rbi]                                    # [16]

        xbh = np.zeros((P, XW), dtype=np.float16)
        xbh[:P - 1, :NT * NB] = xT[featPT].reshape(P - 1, NT * NB)
        xbh[:H, NT * NB:] = xT[mixf]

        prh = np.zeros((P, PRC), dtype=np.float32)
        rk = rowk[keep][:, 0]
        prh[:P - 1, 0:H] = w1[rk]
        prh[:P - 1, H:2 * H] = b1[rk]
        prh[:P - 1, 2 * H:3 * H] = w2[rk]
        prh[:P - 1, 3 * H] = b2[rk]
        mk = idxk[mixf]
        prh[:H, MO:MO + H] = w1[mk]
        prh[:H, MO + H:MO + 2 * H] = b1[mk]
        prh[:H, MO + 2 * H:MO + 3 * H] = w2[mk]
        prh[:H, MO + 3 * H] = b2[mk]

        Wgh = np.zeros((P, NT * KO), dtype=np.float16)
        Wgh[:P - 1] = W16t[featPT].reshape(P - 1, NT * KO)
        Wmh = np.ascontiguousarray(W16t[mixf].reshape(H, KO))

        in_maps.append({"Wg": Wgh, "Wm": Wmh, "xb": xbh, "pr": prh})

    nc = _build()
    res = run_bass_kernel_spmd(nc, in_maps, list(range(NCORES)), trace=TRACE)
    LAST_RESULT = res
    outv = np.zeros((B, O), dtype=np.float32)
    for c in range(NCORES):
        oc = res.results[c]["out"]
        outv += oc[0:NB] + oc[NB:P]
    return outv
